# revision 5
# baseline (speedup 1.0000x reference)
"""Per-sample dynamic-filter Conv2D (VALID, stride 1) on 8 Trainium2 NeuronCores.

Problem: X [16,128,128,128] (NHWC) conv with per-sample filters
kernel [16,3,3,128,128] (HWIO) -> out [16,126,126,128].

Sharding: pure data parallel — 2 samples per core, no communication.

Per-core kernel (per sample):
  1. Transpose X [H*W, Cin] -> X^T [Cin, H*W] via TensorE is_transpose matmuls
     (f32r dtype: 1.5 cyc/row), PSUM->SBUF copies on DVE.
  2. Conv as 9 accumulated matmuls per output tile: out'[co, p] for ALL
     p = oh*W+ow (full-width rows, garbage at ow >= OW discarded later).
     lhsT = filter tap [ci, co] (natural layout), rhs = X^T[:, p + dy*W + dx]
     window (contiguous), f32r full rate at N=512, PSUM accumulation over taps.
  3. Transpose out' [co, 128-chunk] -> [p, co] via TensorE; each 128-chunk is
     exactly one output row (W=128); DMA rows 0:OW -> DRAM NHWC.
"""

import sys

_BASS_PATH = "/opt/trn_rl_repo"
if _BASS_PATH not in sys.path:
    sys.path.insert(0, _BASS_PATH)

import numpy as np

import concourse.bass as bass  # noqa: E402
import concourse.mybir as mybir  # noqa: E402
import concourse.tile as tile  # noqa: E402
from concourse import bacc  # noqa: E402
from concourse.masks import make_identity  # noqa: E402

F32 = mybir.dt.float32
F32R = mybir.dt.float32r

# Full-problem constants
B, H, W, CIN, COUT, KH, KW = 16, 128, 128, 128, 128, 3, 3
N_CORES = 8
S = B // N_CORES  # samples per core


def build_conv_nc(S, H, W, C, CO, KH, KW, n_tile=512):
    """Build the per-core Bass program. Returns compiled nc.

    Requires W == 128 (output-row <-> transpose-chunk alignment), C == 128,
    CO == 128, (H*W) % 128 == 0.
    """
    P = 128
    assert W == P and C == P and CO == P and (H * W) % P == 0
    OH, OW = H - KH + 1, W - KW + 1
    HW = H * W                      # input positions
    NHW = OH * W                    # full-width output positions
    NT = (NHW + n_tile - 1) // n_tile  # output tiles per sample
    # X^T columns: pad past HW so tap-shifted windows stay in bounds
    max_read = (NT - 1) * n_tile  # start of last tile
    last_n = NHW - max_read
    pad_to = ((HW + (KH - 1) * W + (KW - 1) + P - 1) // P) * P
    XT_COLS = pad_to
    NCHUNK = HW // P               # natural-X chunks per sample

    nc = bacc.Bacc("TRN2", target_bir_lowering=False, debug=False)
    xd = nc.dram_tensor("x", [S, HW, C], F32, kind="ExternalInput").ap()
    kd = nc.dram_tensor("k", [S, KH, KW, C, CO], F32, kind="ExternalInput").ap()
    od = nc.dram_tensor("o", [S, OH, OW, CO], F32, kind="ExternalOutput").ap()

    with tile.TileContext(nc) as tc:
        with (
            tc.tile_pool(name="ident", bufs=1) as ident_pool,
            tc.tile_pool(name="xt", bufs=2) as xt_pool,
            tc.tile_pool(name="xn", bufs=8) as xn_pool,
            tc.tile_pool(name="filt", bufs=2) as filt_pool,
            tc.tile_pool(name="ostage", bufs=4) as ostage_pool,
            tc.tile_pool(name="ochunk", bufs=8) as ochunk_pool,
            tc.tile_pool(name="tpsum", bufs=2, space="PSUM") as tpsum_pool,
            tc.tile_pool(name="acc", bufs=4, space="PSUM") as acc_pool,
            tc.tile_pool(name="opsum", bufs=2, space="PSUM") as opsum_pool,
        ):
            ident_f32 = ident_pool.tile([P, P], F32)
            make_identity(nc, ident_f32[:])
            ident = ident_pool.tile([P, P], F32R)
            nc.vector.tensor_copy(ident[:], ident_f32[:])
            ident_r = ident[:]
            PADW = XT_COLS - HW
            zsrc = ident_pool.tile([P, PADW], F32)
            nc.gpsimd.memset(zsrc[:], 0.0)

            for s in range(S):
                # ---- filter load: filt[ci, tap*CO+co] = kd[s, tap, ci, co]
                filt = filt_pool.tile([P, KH * KW * CO], F32R, tag="filt")
                nc.sync.dma_start(
                    out=filt[:].rearrange("ci (t co) -> ci t co", t=KH * KW),
                    in_=kd[s].rearrange("kh kw ci co -> ci (kh kw) co").bitcast(F32R),
                )

                # ---- phase 1: X^T build
                xt = xt_pool.tile([P, XT_COLS], F32R, tag="xt")
                nc.vector.tensor_copy(xt[:, HW:XT_COLS], zsrc[:])
                for n in range(NCHUNK):
                    xn = xn_pool.tile([P, P], F32R, tag="xn")
                    nc.sync.dma_start(
                        out=xn[:], in_=xd[s, n * P : (n + 1) * P, :].bitcast(F32R)
                    )
                    tp = tpsum_pool.tile([P, P], F32R, tag="tp")
                    nc.tensor.transpose(tp[:], xn[:], ident_r)
                    nc.vector.tensor_copy(xt[:, n * P : (n + 1) * P], tp[:])

                # ---- phase 2+3: conv matmuls + output transpose
                for t in range(NT):
                    base = t * n_tile
                    n = min(n_tile, NHW - base)
                    acc = acc_pool.tile([P, n_tile], F32, tag="acc")
                    for tap in range(KH * KW):
                        dy, dx = divmod(tap, KW)
                        off = base + dy * W + dx
                        nc.tensor.matmul(
                            acc[:, :n],
                            filt[:, tap * CO : (tap + 1) * CO],
                            xt[:, off : off + n],
                            start=(tap == 0),
                            stop=(tap == KH * KW - 1),
                        )
                    ostage = ostage_pool.tile([P, n_tile], F32R, tag="ostage")
                    nc.scalar.copy(ostage[:, :n], acc[:, :n])
                    for c in range(n // P):
                        oh = (base + c * P) // P
                        op = opsum_pool.tile([P, P], F32R, tag="op")
                        nc.tensor.transpose(
                            op[:], ostage[:, c * P : (c + 1) * P], ident_r
                        )
                        oc = ochunk_pool.tile([P, P], F32, tag="oc")
                        nc.vector.tensor_copy(oc[:], op[:].bitcast(F32))
                        nc.sync.dma_start(out=od[s, oh, :, :], in_=oc[:OW, :])

    nc.compile()
    return nc


_NC_CACHE = {}


def _get_nc():
    key = (S, H, W, CIN, COUT, KH, KW)
    if key not in _NC_CACHE:
        _NC_CACHE[key] = build_conv_nc(*key)
    return _NC_CACHE[key]


def kernel(**inputs):
    X = np.ascontiguousarray(np.asarray(inputs["X"], dtype=np.float32))
    K = np.ascontiguousarray(np.asarray(inputs["kernel"], dtype=np.float32))
    assert X.shape == (B, H, W, CIN), X.shape
    assert K.shape == (B, KH, KW, CIN, COUT), K.shape

    from concourse.bass_utils import run_bass_kernel_spmd

    nc = _get_nc()
    Xs = X.reshape(B, H * W, CIN)
    in_maps = [
        {"x": Xs[i * S : (i + 1) * S], "k": K[i * S : (i + 1) * S]}
        for i in range(N_CORES)
    ]
    res = run_bass_kernel_spmd(nc, in_maps, list(range(N_CORES)))
    OH, OW = H - KH + 1, W - KW + 1
    out = np.empty((B, OH, OW, COUT), dtype=np.float32)
    for i in range(N_CORES):
        out[i * S : (i + 1) * S] = res.results[i]["o"]
    return out


# revision 10
# speedup vs baseline: 2.3526x; 2.3526x over previous
"""Per-sample dynamic-filter Conv2D (VALID, stride 1) on 8 Trainium2 NeuronCores.

Problem: X [16,128,128,128] (NHWC) conv with per-sample filters
kernel [16,3,3,128,128] (HWIO) -> out [16,126,126,128].

Sharding: pure data parallel — 2 samples per core, no communication.

Per-core kernel (per sample):
  1. Transpose X [H*W, Cin] -> X^T [Cin, H*W] via TensorE is_transpose matmuls
     (f32r dtype: 1.5 cyc/row), PSUM->SBUF copies on DVE.
  2. Conv as 9 accumulated matmuls per output tile: out'[co, p] for ALL
     p = oh*W+ow (full-width rows, garbage at ow >= OW discarded later).
     lhsT = filter tap [ci, co] (natural layout), rhs = X^T[:, p + dy*W + dx]
     window (contiguous), f32r full rate at N=512, PSUM accumulation over taps.
  3. Transpose out' [co, 128-chunk] -> [p, co] via TensorE; each 128-chunk is
     exactly one output row (W=128); DMA rows 0:OW -> DRAM NHWC.
"""

import sys

_BASS_PATH = "/opt/trn_rl_repo"
if _BASS_PATH not in sys.path:
    sys.path.insert(0, _BASS_PATH)

import numpy as np

import concourse.bass as bass  # noqa: E402
import concourse.mybir as mybir  # noqa: E402
import concourse.tile as tile  # noqa: E402
from concourse import bacc  # noqa: E402
from concourse.masks import make_identity  # noqa: E402

F32 = mybir.dt.float32
F32R = mybir.dt.float32r

# Full-problem constants
B, H, W, CIN, COUT, KH, KW = 16, 128, 128, 128, 128, 3, 3
N_CORES = 8
S = B // N_CORES  # samples per core


def build_conv_nc(S, H, W, C, CO, KH, KW, n_tile=512):
    """Build the per-core Bass program. Returns compiled nc.

    Requires W == 128 (output-row <-> transpose-chunk alignment), C == 128,
    CO == 128, (H*W) % 128 == 0.
    """
    P = 128
    assert W == P and C == P and CO == P and (H * W) % P == 0
    OH, OW = H - KH + 1, W - KW + 1
    HW = H * W                      # input positions
    NHW = OH * W                    # full-width output positions
    NT = (NHW + n_tile - 1) // n_tile  # output tiles per sample
    # X^T columns: pad past HW so tap-shifted windows stay in bounds
    max_read = (NT - 1) * n_tile  # start of last tile
    last_n = NHW - max_read
    pad_to = ((HW + (KH - 1) * W + (KW - 1) + P - 1) // P) * P
    XT_COLS = pad_to
    NCHUNK = HW // P               # natural-X chunks per sample

    nc = bacc.Bacc("TRN2", target_bir_lowering=False, debug=False)
    xd = nc.dram_tensor("x", [S, HW, C], F32, kind="ExternalInput").ap()
    kd = nc.dram_tensor("k", [S, KH, KW, C, CO], F32, kind="ExternalInput").ap()
    od = nc.dram_tensor("o", [S, OH, OW, CO], F32, kind="ExternalOutput").ap()

    with tile.TileContext(nc) as tc:
        with (
            tc.tile_pool(name="ident", bufs=1) as ident_pool,
            tc.tile_pool(name="xt", bufs=2) as xt_pool,
            tc.tile_pool(name="xn", bufs=3) as xn_pool,
            tc.tile_pool(name="filt", bufs=2) as filt_pool,
            tc.tile_pool(name="ostage", bufs=4) as ostage_pool,
            tc.tile_pool(name="ochunk", bufs=4) as ochunk_pool,
            tc.tile_pool(name="tpsum", bufs=2, space="PSUM") as tpsum_pool,
            tc.tile_pool(name="acc", bufs=4, space="PSUM") as acc_pool,
            tc.tile_pool(name="opsum", bufs=2, space="PSUM") as opsum_pool,
        ):
            ident_f32 = ident_pool.tile([P, P], F32)
            make_identity(nc, ident_f32[:])
            ident = ident_pool.tile([P, P], F32R)
            nc.vector.tensor_copy(ident[:], ident_f32[:])
            ident_r = ident[:]
            PADW = XT_COLS - HW
            zsrc = ident_pool.tile([P, PADW], F32)
            nc.gpsimd.memset(zsrc[:], 0.0)

            for s in range(S):
                # ---- filter load: filt[ci, tap*CO+co] = kd[s, tap, ci, co]
                filt = filt_pool.tile([P, KH * KW * CO], F32R, tag="filt")
                nc.sync.dma_start(
                    out=filt[:].rearrange("ci (t co) -> ci t co", t=KH * KW),
                    in_=kd[s].rearrange("kh kw ci co -> ci (kh kw) co").bitcast(F32R),
                )

                # ---- phase 1: X^T build (loads batched G chunks per DMA)
                G = min(16, NCHUNK)
                xt = xt_pool.tile([P, XT_COLS], F32R, tag="xt")
                nc.vector.tensor_copy(xt[:, HW:XT_COLS], zsrc[:])
                for g0 in range(0, NCHUNK, G):
                    xn = xn_pool.tile([P, G * P], F32R, tag="xn")
                    # src (p, g, ci) order to match dest free layout [g, ci]
                    src = xd[s, g0 * P : (g0 + G) * P, :].rearrange(
                        "(g p) ci -> p g ci", g=G
                    )
                    nc.sync.dma_start(out=xn[:], in_=src.bitcast(F32R))
                    for g in range(G):
                        n = g0 + g
                        tp = tpsum_pool.tile([P, P], F32R, tag="tp")
                        nc.tensor.transpose(tp[:], xn[:, g * P : (g + 1) * P], ident_r)
                        nc.vector.tensor_copy(xt[:, n * P : (n + 1) * P], tp[:])

                # ---- phase 2+3: conv matmuls + output transpose
                for t in range(NT):
                    base = t * n_tile
                    n = min(n_tile, NHW - base)
                    acc = acc_pool.tile([P, n_tile], F32, tag="acc")
                    for tap in range(KH * KW):
                        dy, dx = divmod(tap, KW)
                        off = base + dy * W + dx
                        nc.tensor.matmul(
                            acc[:, :n],
                            filt[:, tap * CO : (tap + 1) * CO],
                            xt[:, off : off + n],
                            start=(tap == 0),
                            stop=(tap == KH * KW - 1),
                        )
                    ostage = ostage_pool.tile([P, n_tile], F32R, tag="ostage")
                    nc.scalar.copy(ostage[:, :n], acc[:, :n])
                    nchunks = n // P
                    oc = ochunk_pool.tile([P, n_tile], F32, tag="oc")
                    for c in range(nchunks):
                        op = opsum_pool.tile([P, P], F32R, tag="op")
                        nc.tensor.transpose(
                            op[:], ostage[:, c * P : (c + 1) * P], ident_r
                        )
                        nc.vector.tensor_copy(
                            oc[:, c * P : (c + 1) * P], op[:].bitcast(F32)
                        )
                    oh0 = base // P
                    # dest (p, c, co) iteration: od[s, oh0+c, p, co]
                    dst = od[s].rearrange("oh ow co -> ow oh co")[
                        :, oh0 : oh0 + nchunks, :
                    ]
                    nc.scalar.dma_start(
                        out=dst,
                        in_=oc[:OW, : nchunks * P].rearrange(
                            "p (c co) -> p c co", c=nchunks
                        ),
                    )

    nc.compile()
    return nc


_NC_CACHE = {}


def _get_nc():
    key = (S, H, W, CIN, COUT, KH, KW)
    if key not in _NC_CACHE:
        _NC_CACHE[key] = build_conv_nc(*key)
    return _NC_CACHE[key]


def kernel(**inputs):
    X = np.ascontiguousarray(np.asarray(inputs["X"], dtype=np.float32))
    K = np.ascontiguousarray(np.asarray(inputs["kernel"], dtype=np.float32))
    assert X.shape == (B, H, W, CIN), X.shape
    assert K.shape == (B, KH, KW, CIN, COUT), K.shape

    from concourse.bass_utils import run_bass_kernel_spmd

    nc = _get_nc()
    Xs = X.reshape(B, H * W, CIN)
    in_maps = [
        {"x": Xs[i * S : (i + 1) * S], "k": K[i * S : (i + 1) * S]}
        for i in range(N_CORES)
    ]
    res = run_bass_kernel_spmd(nc, in_maps, list(range(N_CORES)))
    OH, OW = H - KH + 1, W - KW + 1
    out = np.empty((B, OH, OW, COUT), dtype=np.float32)
    for i in range(N_CORES):
        out[i * S : (i + 1) * S] = res.results[i]["o"]
    return out


# revision 14
# speedup vs baseline: 2.3535x; 1.0004x over previous
"""Per-sample dynamic-filter Conv2D (VALID, stride 1) on 8 Trainium2 NeuronCores.

Problem: X [16,128,128,128] (NHWC) conv with per-sample filters
kernel [16,3,3,128,128] (HWIO) -> out [16,126,126,128].

Sharding: pure data parallel — 2 samples per core, no communication.

Per-core kernel (per sample):
  1. Transpose X [H*W, Cin] -> X^T [Cin, H*W] via TensorE is_transpose matmuls
     (f32r dtype: 1.5 cyc/row), PSUM->SBUF copies on DVE.
  2. Conv as 9 accumulated matmuls per output tile: out'[co, p] for ALL
     p = oh*W+ow (full-width rows, garbage at ow >= OW discarded later).
     lhsT = filter tap [ci, co] (natural layout), rhs = X^T[:, p + dy*W + dx]
     window (contiguous), f32r full rate at N=512, PSUM accumulation over taps.
  3. Transpose out' [co, 128-chunk] -> [p, co] via TensorE; each 128-chunk is
     exactly one output row (W=128); DMA rows 0:OW -> DRAM NHWC.
"""

import sys

_BASS_PATH = "/opt/trn_rl_repo"
if _BASS_PATH not in sys.path:
    sys.path.insert(0, _BASS_PATH)

import numpy as np

import concourse.bass as bass  # noqa: E402
import concourse.mybir as mybir  # noqa: E402
import concourse.tile as tile  # noqa: E402
from concourse import bacc  # noqa: E402
from concourse.masks import make_identity  # noqa: E402

F32 = mybir.dt.float32
F32R = mybir.dt.float32r

# Full-problem constants
B, H, W, CIN, COUT, KH, KW = 16, 128, 128, 128, 128, 3, 3
N_CORES = 8
S = B // N_CORES  # samples per core


def build_conv_nc(S, H, W, C, CO, KH, KW, n_tile=512):
    """Build the per-core Bass program. Returns compiled nc.

    Requires W == 128 (output-row <-> transpose-chunk alignment), C == 128,
    CO == 128, (H*W) % 128 == 0.
    """
    P = 128
    assert W == P and C == P and CO == P and (H * W) % P == 0
    OH, OW = H - KH + 1, W - KW + 1
    HW = H * W                      # input positions
    NHW = OH * W                    # full-width output positions
    NT = (NHW + n_tile - 1) // n_tile  # output tiles per sample
    # X^T columns: pad past HW so tap-shifted windows stay in bounds
    max_read = (NT - 1) * n_tile  # start of last tile
    last_n = NHW - max_read
    pad_to = ((HW + (KH - 1) * W + (KW - 1) + P - 1) // P) * P
    XT_COLS = pad_to
    NCHUNK = HW // P               # natural-X chunks per sample

    nc = bacc.Bacc("TRN2", target_bir_lowering=False, debug=False)
    xd = nc.dram_tensor("x", [S, HW, C], F32, kind="ExternalInput").ap()
    kd = nc.dram_tensor("k", [S, KH, KW, C, CO], F32, kind="ExternalInput").ap()
    od = nc.dram_tensor("o", [S, OH, OW, CO], F32, kind="ExternalOutput").ap()

    with tile.TileContext(nc) as tc:
        with (
            tc.tile_pool(name="ident", bufs=1) as ident_pool,
            tc.tile_pool(name="xt", bufs=2) as xt_pool,
            tc.tile_pool(name="xn", bufs=4) as xn_pool,
            tc.tile_pool(name="filt", bufs=2) as filt_pool,
            tc.tile_pool(name="ostage", bufs=4) as ostage_pool,
            tc.tile_pool(name="ochunk", bufs=2) as ochunk_pool,
            tc.tile_pool(name="tpsum", bufs=2, space="PSUM") as tpsum_pool,
            tc.tile_pool(name="acc", bufs=4, space="PSUM") as acc_pool,
            tc.tile_pool(name="opsum", bufs=2, space="PSUM") as opsum_pool,
        ):
            ident_f32 = ident_pool.tile([P, P], F32)
            make_identity(nc, ident_f32[:])
            ident = ident_pool.tile([P, P], F32R)
            nc.vector.tensor_copy(ident[:], ident_f32[:])
            ident_r = ident[:]
            PADW = XT_COLS - HW
            zsrc = ident_pool.tile([P, PADW], F32)
            nc.gpsimd.memset(zsrc[:], 0.0)

            for s in range(S):
                # ---- filter load: filt[ci, tap*CO+co] = kd[s, tap, ci, co]
                filt = filt_pool.tile([P, KH * KW * CO], F32R, tag="filt")
                nc.sync.dma_start(
                    out=filt[:].rearrange("ci (t co) -> ci t co", t=KH * KW),
                    in_=kd[s].rearrange("kh kw ci co -> ci (kh kw) co").bitcast(F32R),
                )

                # ---- phase 1: X^T build (loads batched G chunks per DMA;
                # transposes batched 4-per-PSUM-bank so one copy drains 4)
                GMAX = min(8, NCHUNK)
                # small first groups on the first sample to start PE sooner
                gsizes = []
                rem = NCHUNK
                for gsz in [4, 4] if s == 0 else []:
                    if rem >= gsz:
                        gsizes.append(gsz)
                        rem -= gsz
                while rem > 0:
                    gsz = min(GMAX, rem)
                    gsizes.append(gsz)
                    rem -= gsz
                xt = xt_pool.tile([P, XT_COLS], F32R, tag="xt")
                nc.vector.tensor_copy(xt[:, HW:XT_COLS], zsrc[:])
                g0 = 0
                for gi, G in enumerate(gsizes):
                    xn = xn_pool.tile([P, GMAX * P], F32R, tag="xn")
                    # src (p, g, ci) order to match dest free layout [g, ci]
                    src = xd[s, g0 * P : (g0 + G) * P, :].rearrange(
                        "(g p) ci -> p g ci", g=G
                    )
                    nc.sync.dma_start(out=xn[:, : G * P], in_=src.bitcast(F32R))
                    for c0 in range(0, G, 4):
                        cn = min(4, G - c0)
                        tp = tpsum_pool.tile([P, 4 * P], F32R, tag="tp")
                        for g in range(c0, c0 + cn):
                            nc.tensor.transpose(
                                tp[:, (g - c0) * P : (g - c0 + 1) * P],
                                xn[:, g * P : (g + 1) * P],
                                ident_r,
                            )
                        n = g0 + c0
                        if (n // 4) % 2 == 0:
                            nc.vector.tensor_copy(
                                xt[:, n * P : (n + cn) * P], tp[:, : cn * P]
                            )
                        else:
                            nc.scalar.copy(
                                xt[:, n * P : (n + cn) * P], tp[:, : cn * P]
                            )
                    g0 += G

                # ---- phase 2+3: conv matmuls + output transpose
                for t in range(NT):
                    base = t * n_tile
                    n = min(n_tile, NHW - base)
                    acc = acc_pool.tile([P, n_tile], F32, tag="acc")
                    for tap in range(KH * KW):
                        dy, dx = divmod(tap, KW)
                        off = base + dy * W + dx
                        nc.tensor.matmul(
                            acc[:, :n],
                            filt[:, tap * CO : (tap + 1) * CO],
                            xt[:, off : off + n],
                            start=(tap == 0),
                            stop=(tap == KH * KW - 1),
                        )
                    ostage = ostage_pool.tile([P, n_tile], F32R, tag="ostage")
                    nc.scalar.copy(ostage[:, :n], acc[:, :n])
                    nchunks = n // P
                    oc = ochunk_pool.tile([P, n_tile], F32, tag="oc")
                    op = opsum_pool.tile([P, n_tile], F32R, tag="op")
                    for c in range(nchunks):
                        nc.tensor.transpose(
                            op[:, c * P : (c + 1) * P],
                            ostage[:, c * P : (c + 1) * P],
                            ident_r,
                        )
                    nc.vector.tensor_copy(
                        oc[:, : nchunks * P], op[:, : nchunks * P].bitcast(F32)
                    )
                    oh0 = base // P
                    # dest (p, c, co) iteration: od[s, oh0+c, p, co]
                    dst = od[s].rearrange("oh ow co -> ow oh co")[
                        :, oh0 : oh0 + nchunks, :
                    ]
                    nc.sync.dma_start(
                        out=dst,
                        in_=oc[:OW, : nchunks * P].rearrange(
                            "p (c co) -> p c co", c=nchunks
                        ),
                    )

    nc.compile()
    return nc


_NC_CACHE = {}


def _get_nc():
    key = (S, H, W, CIN, COUT, KH, KW)
    if key not in _NC_CACHE:
        _NC_CACHE[key] = build_conv_nc(*key)
    return _NC_CACHE[key]


def kernel(**inputs):
    X = np.ascontiguousarray(np.asarray(inputs["X"], dtype=np.float32))
    K = np.ascontiguousarray(np.asarray(inputs["kernel"], dtype=np.float32))
    assert X.shape == (B, H, W, CIN), X.shape
    assert K.shape == (B, KH, KW, CIN, COUT), K.shape

    from concourse.bass_utils import run_bass_kernel_spmd

    nc = _get_nc()
    Xs = X.reshape(B, H * W, CIN)
    in_maps = [
        {"x": Xs[i * S : (i + 1) * S], "k": K[i * S : (i + 1) * S]}
        for i in range(N_CORES)
    ]
    res = run_bass_kernel_spmd(nc, in_maps, list(range(N_CORES)))
    OH, OW = H - KH + 1, W - KW + 1
    out = np.empty((B, OH, OW, COUT), dtype=np.float32)
    for i in range(N_CORES):
        out[i * S : (i + 1) * S] = res.results[i]["o"]
    return out


# revision 18
# speedup vs baseline: 2.4357x; 1.0350x over previous
"""Per-sample dynamic-filter Conv2D (VALID, stride 1) on 8 Trainium2 NeuronCores.

Problem: X [16,128,128,128] (NHWC) conv with per-sample filters
kernel [16,3,3,128,128] (HWIO) -> out [16,126,126,128].

Sharding: pure data parallel — 2 samples per core, no communication.

Per-core kernel (per sample):
  1. Transpose X [H*W, Cin] -> X^T [Cin, H*W] via TensorE is_transpose matmuls
     (f32r dtype: 1.5 cyc/row), PSUM->SBUF copies on DVE.
  2. Conv as 9 accumulated matmuls per output tile: out'[co, p] for ALL
     p = oh*W+ow (full-width rows, garbage at ow >= OW discarded later).
     lhsT = filter tap [ci, co] (natural layout), rhs = X^T[:, p + dy*W + dx]
     window (contiguous), f32r full rate at N=512, PSUM accumulation over taps.
  3. Transpose out' [co, 128-chunk] -> [p, co] via TensorE; each 128-chunk is
     exactly one output row (W=128); DMA rows 0:OW -> DRAM NHWC.
"""

import sys

_BASS_PATH = "/opt/trn_rl_repo"
if _BASS_PATH not in sys.path:
    sys.path.insert(0, _BASS_PATH)

import numpy as np

import concourse.bass as bass  # noqa: E402
import concourse.mybir as mybir  # noqa: E402
import concourse.tile as tile  # noqa: E402
from concourse import bacc  # noqa: E402
from concourse.masks import make_identity  # noqa: E402

F32 = mybir.dt.float32
F32R = mybir.dt.float32r

# Full-problem constants
B, H, W, CIN, COUT, KH, KW = 16, 128, 128, 128, 128, 3, 3
N_CORES = 8
S = B // N_CORES  # samples per core


def build_conv_nc(S, H, W, C, CO, KH, KW, n_tile=512, conv_tmode=False):
    """Build the per-core Bass program. Returns compiled nc.

    Requires W == 128 (output-row <-> transpose-chunk alignment), C == 128,
    CO == 128, (H*W) % 128 == 0.
    """
    P = 128
    assert W == P and C == P and CO == P and (H * W) % P == 0
    OH, OW = H - KH + 1, W - KW + 1
    HW = H * W                      # input positions
    NHW = OH * W                    # full-width output positions
    NT = (NHW + n_tile - 1) // n_tile  # output tiles per sample
    # X^T columns: pad past HW so tap-shifted windows stay in bounds
    max_read = (NT - 1) * n_tile  # start of last tile
    last_n = NHW - max_read
    pad_to = ((HW + (KH - 1) * W + (KW - 1) + P - 1) // P) * P
    XT_COLS = pad_to
    NCHUNK = HW // P               # natural-X chunks per sample

    nc = bacc.Bacc("TRN2", target_bir_lowering=False, debug=False)
    xd = nc.dram_tensor("x", [S, HW, C], F32, kind="ExternalInput").ap()
    kd = nc.dram_tensor("k", [S, KH, KW, C, CO], F32, kind="ExternalInput").ap()
    od = nc.dram_tensor("o", [S, OH, OW, CO], F32, kind="ExternalOutput").ap()

    with tile.TileContext(nc) as tc:
        with (
            tc.tile_pool(name="ident", bufs=1) as ident_pool,
            tc.tile_pool(name="xt", bufs=2) as xt_pool,
            tc.tile_pool(name="xn", bufs=4) as xn_pool,
            tc.tile_pool(name="filt", bufs=2) as filt_pool,
            tc.tile_pool(name="ostage", bufs=4) as ostage_pool,
            tc.tile_pool(name="ochunk", bufs=2) as ochunk_pool,
            tc.tile_pool(name="tpsum", bufs=2, space="PSUM") as tpsum_pool,
            tc.tile_pool(name="acc", bufs=4, space="PSUM") as acc_pool,
            tc.tile_pool(name="opsum", bufs=2, space="PSUM") as opsum_pool,
        ):
            ident_f32 = ident_pool.tile([P, P], F32)
            make_identity(nc, ident_f32[:])
            ident = ident_pool.tile([P, P], F32R)
            nc.vector.tensor_copy(ident[:], ident_f32[:])
            ident_r = ident[:]
            PADW = XT_COLS - HW
            zsrc = ident_pool.tile([P, PADW], F32)
            nc.gpsimd.memset(zsrc[:], 0.0)

            def emit_tile(s, t, filt, xt):
                """Emit one output tile: 9 accumulated MMs + output transpose."""
                base = t * n_tile
                n = min(n_tile, NHW - base)
                acc = acc_pool.tile(
                    [P, n_tile], F32R if conv_tmode else F32, tag="acc", name="acc"
                )
                for tap in range(KH * KW):
                    dy, dx = divmod(tap, KW)
                    off = base + dy * W + dx
                    nc.tensor.matmul(
                        acc[:, :n],
                        filt[:, tap * CO : (tap + 1) * CO],
                        xt[:, off : off + n],
                        start=(tap == 0),
                        stop=(tap == KH * KW - 1),
                        is_transpose=conv_tmode or None,
                    )
                ostage = ostage_pool.tile([P, n_tile], F32R, tag="ostage", name="ostage")
                nc.scalar.copy(ostage[:, :n], acc[:, :n])
                nchunks = n // P
                oc = ochunk_pool.tile([P, n_tile], F32, tag="oc", name="oc")
                op = opsum_pool.tile([P, n_tile], F32R, tag="op", name="op")
                for c in range(nchunks):
                    nc.tensor.transpose(
                        op[:, c * P : (c + 1) * P],
                        ostage[:, c * P : (c + 1) * P],
                        ident_r,
                    )
                nc.vector.tensor_copy(
                    oc[:, : nchunks * P], op[:, : nchunks * P].bitcast(F32)
                )
                oh0 = base // P
                # dest (p, c, co) iteration: od[s, oh0+c, p, co]
                dst = od[s].rearrange("oh ow co -> ow oh co")[:, oh0 : oh0 + nchunks, :]
                nc.sync.dma_start(
                    out=dst,
                    in_=oc[:OW, : nchunks * P].rearrange(
                        "p (c co) -> p c co", c=nchunks
                    ),
                )

            halo = (KH - 1) * W + (KW - 1)
            for s in range(S):
                # ---- filter load: filt[ci, tap*CO+co] = kd[s, tap, ci, co]
                filt = filt_pool.tile([P, KH * KW * CO], F32R, tag="filt")
                nc.sync.dma_start(
                    out=filt[:].rearrange("ci (t co) -> ci t co", t=KH * KW),
                    in_=kd[s].rearrange("kh kw ci co -> ci (kh kw) co").bitcast(F32R),
                )

                # ---- phase 1 (X^T build) with phase 2+3 tiles interleaved as
                # soon as the X^T columns they read have been emitted.
                GMAX = min(8, NCHUNK)
                gsizes = []
                rem = NCHUNK
                for gsz in [4, 4] if s == 0 else []:
                    if rem >= gsz:
                        gsizes.append(gsz)
                        rem -= gsz
                while rem > 0:
                    gsz = min(GMAX, rem)
                    gsizes.append(gsz)
                    rem -= gsz
                xt = xt_pool.tile([P, XT_COLS], F32R, tag="xt")
                nc.vector.tensor_copy(xt[:, HW:XT_COLS], zsrc[:])
                g0 = 0
                next_t = 0
                for gi, G in enumerate(gsizes):
                    xn = xn_pool.tile([P, GMAX * P], F32R, tag="xn")
                    # src (p, g, ci) order to match dest free layout [g, ci]
                    src = xd[s, g0 * P : (g0 + G) * P, :].rearrange(
                        "(g p) ci -> p g ci", g=G
                    )
                    nc.sync.dma_start(out=xn[:, : G * P], in_=src.bitcast(F32R))
                    for c0 in range(0, G, 4):
                        cn = min(4, G - c0)
                        tp = tpsum_pool.tile([P, 4 * P], F32R, tag="tp")
                        for g in range(c0, c0 + cn):
                            nc.tensor.transpose(
                                tp[:, (g - c0) * P : (g - c0 + 1) * P],
                                xn[:, g * P : (g + 1) * P],
                                ident_r,
                            )
                        n = g0 + c0
                        if (n // 4) % 2 == 0:
                            nc.vector.tensor_copy(
                                xt[:, n * P : (n + cn) * P], tp[:, : cn * P]
                            )
                        else:
                            nc.scalar.copy(
                                xt[:, n * P : (n + cn) * P], tp[:, : cn * P]
                            )
                    g0 += G
                    # interleave ready conv tiles (their reads end at
                    # (t+1)*n_tile + halo; ready when within built columns,
                    # or when reads extend only into the zero pad)
                    cols_done = g0 * P
                    while next_t < NT and (
                        (next_t + 1) * n_tile + halo <= cols_done
                        or cols_done >= HW
                    ):
                        emit_tile(s, next_t, filt, xt)
                        next_t += 1
                for t in range(next_t, NT):
                    emit_tile(s, t, filt, xt)

    nc.compile()
    return nc


_NC_CACHE = {}


def _get_nc():
    key = (S, H, W, CIN, COUT, KH, KW)
    if key not in _NC_CACHE:
        _NC_CACHE[key] = build_conv_nc(*key)
    return _NC_CACHE[key]


def kernel(**inputs):
    X = np.ascontiguousarray(np.asarray(inputs["X"], dtype=np.float32))
    K = np.ascontiguousarray(np.asarray(inputs["kernel"], dtype=np.float32))
    assert X.shape == (B, H, W, CIN), X.shape
    assert K.shape == (B, KH, KW, CIN, COUT), K.shape

    from concourse.bass_utils import run_bass_kernel_spmd

    nc = _get_nc()
    Xs = X.reshape(B, H * W, CIN)
    in_maps = [
        {"x": Xs[i * S : (i + 1) * S], "k": K[i * S : (i + 1) * S]}
        for i in range(N_CORES)
    ]
    res = run_bass_kernel_spmd(nc, in_maps, list(range(N_CORES)))
    OH, OW = H - KH + 1, W - KW + 1
    out = np.empty((B, OH, OW, COUT), dtype=np.float32)
    for i in range(N_CORES):
        out[i * S : (i + 1) * S] = res.results[i]["o"]
    return out
